# revision 26
# baseline (speedup 1.0000x reference)
"""Trainium2 Bass kernel for a dual-stream "DifAttention" block.

Work partitioning: per batch element b the module needs two outputs
  out_x[b] = (attend(qx,kx,vx) + attend(qyo,kx,vx,neg)) @ Wp^T + bp
  out_y[b] = (attend(qy,ky,vx) + attend(qxo,ky,vx,neg)) @ Wp^T + bp
With B=4 this is 8 fully independent (batch, stream) units -> one per core,
no collectives.  Each core runs the same SPMD program on inputs
  t_qk (source of q,k), t_v (source of v), t_qo (source of the cross query).

Schedule: ScalarE's exp stream (24 x 1M softmax elements = ~18.4us/unit,
~220us total) is the pacing engine, so the emission interleaves everything
under it:
  - prologue projects only K-col0/Q-col0 so the first exp lands ~10us in;
  - all remaining projection work (K/Q cols 1-5, V m-tiles, QO cols) is cut
    into ~12-matmul "filler" groups popped one per exp slot inside the
    attention units, keeping the PE dense (HAM clock gate stays warm);
  - PSUM: psS (2 bufs x [128,1024] = 4 banks) holds S^T for a head pair
    (h0 cols 0-511, h64 cols 512-1023, one n-chunk), double-buffered so
    ScalarE never waits; psO (2 bufs = 4 banks) holds the AV accumulators.
  - softmax denominators (ones-column row 64 of the AV output) are
    reciprocal'd in place in PSUM with the fast approx DVE op, DMA-bounced
    through DRAM for the partition broadcast, and applied by DVE multiplies
    reading PSUM directly (no ScalarE staging copies).
  - output projection streams per n-tile through 4 rotating PSUM tiles at
    the tail and DMAs straight from PSUM to DRAM.
"""

import numpy as np
import ml_dtypes

import concourse.bass as bass
import concourse.bacc as bacc
import concourse.tile as tile
from concourse import mybir
from concourse.bass_utils import run_bass_kernel_spmd

P = 128
B, N, C = 4, 1024, 768
H, HD = 12, 64
CT = C // P           # 6 column tiles (= head pairs)
NT = N // P           # 8 sequence tiles
NU = 2 * CT           # 12 attention units per core (6 self + 6 cross)
SCALE = HD ** -0.5    # 0.125

FP32 = mybir.dt.float32
BF16 = mybir.dt.bfloat16
EXP = mybir.ActivationFunctionType.Exp


def build_kernel():
    nc = bacc.Bacc("TRN2", target_bir_lowering=False, debug=False, num_devices=8)

    d_qk = nc.dram_tensor("qkT", [C, N], BF16, kind="ExternalInput")
    d_v = nc.dram_tensor("vT", [C, N], BF16, kind="ExternalInput")
    d_qo = nc.dram_tensor("qoT", [C, N], BF16, kind="ExternalInput")
    d_wq = nc.dram_tensor("wqT", [C, C], BF16, kind="ExternalInput")
    d_wk = nc.dram_tensor("wkT", [C, C], BF16, kind="ExternalInput")
    d_wv = nc.dram_tensor("wvT", [C, C], BF16, kind="ExternalInput")
    d_wqo = nc.dram_tensor("wqoT", [C, C], BF16, kind="ExternalInput")
    d_wp = nc.dram_tensor("wpT", [C, C], BF16, kind="ExternalInput")
    d_out = nc.dram_tensor("out", [N, C], FP32, kind="ExternalOutput")
    # scratch for the reciprocal-row partition broadcast (psum -> dram -> bcast)
    d_scr = nc.dram_tensor("scr", [NU, 2, N], FP32)

    with tile.TileContext(nc) as tc:
        _body(tc, d_qk, d_v, d_qo, d_wq, d_wk, d_wv, d_wqo, d_wp, d_out, d_scr)
    nc.compile()
    return nc


def _body(tc, d_qk, d_v, d_qo, d_wq, d_wk, d_wv, d_wqo, d_wp, d_out, d_scr):
    nc = tc.nc
    from contextlib import ExitStack
    ctx = ExitStack()
    persist = ctx.enter_context(tc.tile_pool(name="persist", bufs=1))
    xpool = ctx.enter_context(tc.tile_pool(name="xpool", bufs=1))
    apool = ctx.enter_context(tc.tile_pool(name="apool", bufs=2))
    wpool = ctx.enter_context(tc.tile_pool(name="wpool", bufs=2))
    npool = ctx.enter_context(tc.tile_pool(name="npool", bufs=2))
    psS = ctx.enter_context(tc.tile_pool(name="psS", bufs=2, space="PSUM"))
    psO = ctx.enter_context(tc.tile_pool(name="psO", bufs=2, space="PSUM"))

    # ---- persistent tensors -------------------------------------------------
    qt = persist.tile([P, CT, N], BF16, name="qt")
    kt = persist.tile([P, CT, N], BF16, name="kt")
    qot = persist.tile([P, CT, N], BF16, name="qot")
    vsb = persist.tile([P, NT, H, HD + 1], BF16, name="vsb")
    onorm = persist.tile([P, CT, N], BF16, name="onorm")

    # warm the exp table while the first DMAs land
    warm = npool.tile([1, 8], FP32, tag="warm", bufs=1, name="warm")
    nc.vector.memset(warm[:], 0.0)
    nc.scalar.activation(warm[:], warm[:], EXP)
    nc.vector.memset(vsb[:, :, :, HD:HD + 1], 1.0)

    # ---- input loads, split across the two DMA queues ----------------------
    # xqk/wk arrive first (K-col0 is the critical path to the first exp)
    xqk = xpool.tile([P, CT, N], BF16, tag="x", bufs=2, name="xqk")
    xv = xpool.tile([P, CT, N], BF16, tag="x", bufs=2, name="xv")
    wk = wpool.tile([P, CT, C], BF16, tag="w", name="wk")
    wq = wpool.tile([P, CT, C], BF16, tag="w", name="wq")
    xqk_src = d_qk.ap().rearrange("(t p) n -> p t n", p=P)
    nc.scalar.dma_start(wk[:], d_wk.ap().rearrange("(t p) co -> p t co", p=P))
    nc.sync.dma_start(xqk[:, 0:3, :], xqk_src[:, 0:3, :])
    nc.gpsimd.dma_start(xqk[:, 3:CT, :], xqk_src[:, 3:CT, :])
    nc.scalar.dma_start(wq[:], d_wq.ap().rearrange("(t p) co -> p t co", p=P))
    nc.sync.dma_start(xv[:], d_v.ap().rearrange("(t p) n -> p t n", p=P))

    # keep the PE array busy (HAM clock gate warm) while the loads land
    wmm = npool.tile([P, 512], BF16, tag="t", bufs=1, name="wmm")
    nc.vector.memset(wmm[:], 0.25)
    for i in range(2):
        pw = psS.tile([P, N], FP32, tag="s", name="ps_warm")
        for k in range(8):
            nc.tensor.matmul(pw[:, 0:512], wmm[:, 0:P], wmm[:],
                             start=True, stop=True)

    # ---- projection group emitters -----------------------------------------
    def qkv_col(wsb, src, dst, co):
        # one column tile (head pair co) of a q/k/qo projection: 12 matmuls
        ps = psS.tile([P, N], FP32, tag="s", name="ps_qkv")
        for ch in range(2):
            nsl = slice(ch * 512, (ch + 1) * 512)
            for ct in range(CT):
                nc.tensor.matmul(
                    ps[:, nsl],
                    wsb[:, ct, co * P:(co + 1) * P],
                    src[:, ct, nsl],
                    start=(ct == 0), stop=(ct == CT - 1))
        nc.vector.tensor_copy(dst[:, co, :], ps[:])

    def v_mt(wv, mt):
        # one m-tile of the V projection: 12 matmuls
        ps = psS.tile([P, N], FP32, tag="s", name="ps_v")
        for base, wd in ((0, 512), (512, 256)):
            for ct in range(CT):
                nc.tensor.matmul(
                    ps[:, base:base + wd],
                    xv[:, ct, mt * P:(mt + 1) * P],
                    wv[:, ct, base:base + wd],
                    start=(ct == 0), stop=(ct == CT - 1))
        nc.vector.tensor_copy(
            vsb[:, mt, :, 0:HD],
            ps[:, 0:C].rearrange("p (h d) -> p h d", h=H))

    # ---- filler queue -------------------------------------------------------
    # Later-phase tensors reuse earlier slots; each load is emitted only
    # after every read of the slot's previous occupant has been emitted.
    wv = [None]
    wqo = [None]
    wp = [None]
    xqo = [None]

    def load_wv():
        # own slot: K-col reads of wk are still being emitted after this
        wv[0] = wpool.tile([P, CT, C], BF16, tag="wv", bufs=1, name="wv")
        nc.sync.dma_start(wv[0][:],
                          d_wv.ap().rearrange("(t p) co -> p t co", p=P))

    def load_late():
        # Overwrite in place (plain WAR deps, no pool realloc): xqk takes the
        # cross-query activations (all K/Q cols emitted by now), wv takes
        # wqo (all V m-tiles emitted), wk takes wp (all K cols emitted).
        nc.sync.dma_start(xqk[:], d_qo.ap().rearrange("(t p) n -> p t n", p=P))
        nc.gpsimd.dma_start(wv[0][:],
                            d_wqo.ap().rearrange("(t p) co -> p t co", p=P))
        nc.sync.dma_start(wk[:], d_wp.ap().rearrange("(t p) co -> p t co", p=P))
        xqo[0] = xqk
        wqo[0] = wv[0]
        wp[0] = wk

    fillers = []
    fillers.append((("K", 1), lambda: qkv_col(wk, xqk, kt, 1)))
    fillers.append((("Q", 1), lambda: qkv_col(wq, xqk, qt, 1)))
    fillers.append((("V", 0), lambda: (load_wv(), v_mt(wv[0], 0))))
    for m in range(1, NT):
        fillers.append((("V", m), lambda m=m: v_mt(wv[0], m)))
    for co in range(2, CT):
        fillers.append((("K", co), lambda co=co: qkv_col(wk, xqk, kt, co)))
        fillers.append((("Q", co), lambda co=co: qkv_col(wq, xqk, qt, co)))
    fillers.append((("LOADS", 0), load_late))
    for co in range(CT):
        fillers.append((("QO", co),
                        lambda co=co: qkv_col(wqo[0], xqo[0], qot, co)))

    emitted = set()
    gslot = [0]

    def pop_filler():
        key, fn = fillers.pop(0)
        fn()
        emitted.add(key)

    def need(key):
        while key not in emitted:
            pop_filler()

    def pace_fillers():
        # eager 1-per-slot for projection work the early units depend on;
        # QO columns wait for their DMAs (~slot 28) and spread out to keep
        # the PE dense through the middle units
        gslot[0] += 1
        if not fillers:
            return
        if fillers[0][0][0] != "QO":
            pop_filler()
        elif gslot[0] >= 30 and gslot[0] % 2 == 0:
            pop_filler()

    # ---- attention unit machinery ------------------------------------------
    def emit_av_chunk(pend, mt):
        # 4 AV matmuls (m-tile mt of all four (head, ch) accumulations)
        pp, att, a, po = pend["p"], pend["att"], pend["a"], pend["po"]
        need(("V", mt))
        if po[0] is None:
            po[0] = psO.tile([HD + 1, N], FP32, tag="o", name="po1")
            po[1] = psO.tile([HD + 1, N], FP32, tag="o", name="po2")
        for h in range(2):
            for ch in range(2):
                nc.tensor.matmul(
                    po[h][:, ch * 512:(ch + 1) * 512],
                    vsb[:, mt, 2 * pp + h, :],
                    a[:, mt, ch * 1024 + h * 512: ch * 1024 + h * 512 + 512],
                    start=(mt == 0), stop=(mt == NT - 1),
                    skip_group_check=True)

    def emit_norm(pend):
        u, p, att, po = pend["u"], pend["p"], pend["att"], pend["po"]
        # Ordered so no DVE op ever head-of-line-blocks on a DMA that a
        # later-emitted DVE op doesn't also need: fire both denominator-row
        # bounces first, stage the AV accumulators to SBUF (freeing psO),
        # then broadcasts, reciprocals, and the normalize multiplies.
        dsbs = []
        for j in (0, 1):
            # denominator row -> fp32 SBUF -> DRAM (partition broadcast
            # source); recip happens at base 0 on the broadcast tile since
            # custom DVE ops silently no-op at non-zero base partitions
            dsb = npool.tile([HD + 1, N], FP32, tag="dsb", bufs=2, name="dsb")
            nc.vector.tensor_copy(dsb[HD:HD + 1, :], po[j][HD:HD + 1, :])
            nc.gpsimd.dma_start(d_scr.ap()[u, j, :], dsb[HD:HD + 1, :])
            dsbs.append(dsb)
        stage = npool.tile([HD + 1, 2, N], BF16, tag="stg", bufs=2,
                           name="stage")
        nc.vector.tensor_copy(stage[:, 0, :], po[0][:])
        nc.vector.tensor_copy(stage[:, 1, :], po[1][:])
        rs = []
        for j in (0, 1):
            r = npool.tile([HD, N], FP32, tag="r", bufs=2, name="r_att")
            srow = d_scr.ap()[u, j:j + 1, :]
            bcast = bass.AP(tensor=srow.tensor, offset=srow.offset,
                            ap=[[0, HD]] + list(srow.ap[1:]))
            nc.gpsimd.dma_start(r[:], bcast)
            rs.append(r)
        nc.vector.reciprocal_approx_fast(rs[1][:], rs[1][:])
        nc.vector.reciprocal_approx_fast(rs[0][:], rs[0][:])
        # j=1 (head B) first: its partition-shift DMA is the longest hop
        t1 = npool.tile([HD, N], BF16, tag="t", bufs=1, name="t1")
        nc.vector.tensor_mul(t1[:], stage[0:HD, 1, :], rs[1][:])
        if att == 0:
            nc.gpsimd.dma_start(onorm[HD:P, p, :], t1[:])
            nc.vector.tensor_mul(onorm[0:HD, p, :], stage[0:HD, 0, :],
                                 rs[0][:])
        else:
            ts = npool.tile([P, N], BF16, tag="out", bufs=2, name="ts")
            nc.gpsimd.dma_start(ts[HD:P, :], t1[:])
            t0 = npool.tile([HD, N], BF16, tag="t", bufs=1, name="t0")
            nc.vector.tensor_mul(t0[:], stage[0:HD, 0, :], rs[0][:])
            nc.vector.tensor_add(onorm[0:HD, p, :], onorm[0:HD, p, :],
                                 t0[:])
            nc.vector.tensor_add(onorm[HD:P, p, :], onorm[HD:P, p, :],
                                 ts[HD:P, :])

    # ---- prologue: K-col0, Q-col0 so the exp stream starts early ------------
    qkv_col(wk, xqk, kt, 0)
    emitted.add(("K", 0))
    qkv_col(wq, xqk, qt, 0)
    emitted.add(("Q", 0))

    # ---- main software-pipelined loop ---------------------------------------
    units = [(p, 0) for p in range(CT)] + [(p, 1) for p in range(CT)]
    pend = None
    for ui, (p, att) in enumerate(units):
        if att == 0:
            need(("K", p))
            need(("Q", p))
        else:
            need(("QO", p))
        last = ui == NU - 1
        qsrc = qt if att == 0 else qot
        sgn = SCALE if att == 0 else -SCALE
        a = apool.tile([P, NT, 2048], BF16, tag="a", name="a_att")
        norm_mid = False
        for mt in range(NT):
            msl = slice(mt * P, (mt + 1) * P)
            for ch in range(2):
                nsl = slice(ch * 512, (ch + 1) * 512)
                s = psS.tile([P, N], FP32, tag="s", name="s_att")
                nc.tensor.matmul(s[:, 0:512], kt[0:HD, p, msl],
                                 qsrc[0:HD, p, nsl], start=True, stop=True)
                nc.tensor.matmul(s[:, 512:1024], kt[HD:P, p, msl],
                                 qsrc[HD:P, p, nsl], start=True, stop=True)
                nc.scalar.activation(
                    a[:, mt, ch * 1024:(ch + 1) * 1024], s[:], EXP, scale=sgn)
            if pend is not None:
                if last:
                    # 2 AV chunks per slot so the second-to-last unit's norm
                    # chain starts early and finishes before the drain
                    if mt < 4:
                        emit_av_chunk(pend, 2 * mt)
                        emit_av_chunk(pend, 2 * mt + 1)
                    if mt == 3:
                        emit_norm(pend)
                        norm_mid = True
                else:
                    emit_av_chunk(pend, mt)
            pace_fillers()
        if pend is not None and not norm_mid:
            emit_norm(pend)
        pend = {"u": ui, "p": p, "att": att, "a": a, "po": [None, None]}

    # ---- drain: AV + norm of the last unit ----------------------------------
    while fillers:
        pop_filler()
    for mt in range(NT):
        emit_av_chunk(pend, mt)
    emit_norm(pend)

    # ---- output projection ---------------------------------------------------
    # nt0/nt1 run their ct0-4 partials while the drain norm chain completes.
    def proj_partial(ps, nt, cts, start, stop):
        for base, wd in ((0, 512), (512, 256)):
            for ct in cts:
                nc.tensor.matmul(
                    ps[:, base:base + wd],
                    onorm[:, ct, nt * P:(nt + 1) * P],
                    wp[0][:, ct, base:base + wd],
                    start=(start and ct == cts[0]),
                    stop=(stop and ct == cts[-1]))

    def proj_store(ps, nt):
        osb = npool.tile([P, C], FP32, tag="out", bufs=2, name="osb")
        nc.vector.tensor_copy(osb[:], ps[:, 0:C])
        nc.sync.dma_start(d_out.ap()[nt * P:(nt + 1) * P, :], osb[:])

    pres = []
    for nt in range(4):
        pool = psS if nt < 2 else psO
        ps = pool.tile([P, N], FP32, tag="s" if pool is psS else "o",
                       name="ps_proj")
        proj_partial(ps, nt, list(range(CT - 1)), start=True, stop=False)
        pres.append(ps)
    for nt in range(4):
        proj_partial(pres[nt], nt, [CT - 1], start=False, stop=True)
        proj_store(pres[nt], nt)
    for nt in range(4, NT):
        pool = psS if nt % 2 == 0 else psO
        ps = pool.tile([P, N], FP32, tag="s" if pool is psS else "o",
                       name="ps_proj")
        proj_partial(ps, nt, list(range(CT)), start=True, stop=True)
        proj_store(ps, nt)

    ctx.close()


_NC = None


def _get_nc():
    global _NC
    if _NC is None:
        _NC = build_kernel()
    return _NC


def prepare_in_maps(x, y, w_qkv, w_proj, b_proj):
    x = np.asarray(x, np.float32)
    y = np.asarray(y, np.float32)
    w_qkv = np.asarray(w_qkv, np.float32)
    w_proj = np.asarray(w_proj, np.float32)

    bf = ml_dtypes.bfloat16
    cbf = lambda a: np.ascontiguousarray(a.T).astype(bf)
    wqoT = cbf(w_qkv[0:C])
    wqT = cbf(w_qkv[C:2 * C])
    wkT = cbf(w_qkv[2 * C:3 * C])
    wvT = cbf(w_qkv[3 * C:4 * C])
    wpT = np.ascontiguousarray(w_proj.T).astype(bf)

    in_maps = []
    for i in range(8):
        b = i % 4
        isx = i < 4
        t_qk = x[b] if isx else y[b]
        t_qo = y[b] if isx else x[b]
        in_maps.append({
            "qkT": cbf(t_qk), "vT": cbf(x[b]), "qoT": cbf(t_qo),
            "wqT": wqT, "wkT": wkT, "wvT": wvT, "wqoT": wqoT,
            "wpT": wpT,
        })
    return in_maps


def kernel(x, y, w_qkv, w_proj, b_proj):
    nc = _get_nc()
    in_maps = prepare_in_maps(x, y, w_qkv, w_proj, b_proj)
    res = run_bass_kernel_spmd(nc, in_maps, list(range(8)))
    bpf = np.asarray(b_proj, np.float32)
    out_x = np.stack([res.results[b]["out"] for b in range(4)]) + bpf
    out_y = np.stack([res.results[4 + b]["out"] for b in range(4)]) + bpf
    return out_x.astype(np.float32), out_y.astype(np.float32)


if __name__ == "__main__":
    rng = np.random.default_rng(0)
    ins = {
        "x": rng.standard_normal((B, N, C), dtype=np.float32),
        "y": rng.standard_normal((B, N, C), dtype=np.float32),
        "w_qkv": (rng.standard_normal((4 * C, C)) * 0.02).astype(np.float32),
        "w_proj": (rng.standard_normal((C, C)) * 0.02).astype(np.float32),
        "b_proj": (rng.standard_normal(C) * 0.02).astype(np.float32),
    }
    ox, oy = kernel(**ins)
    print(ox.shape, oy.shape, ox.dtype)
